# revision 1
# baseline (speedup 1.0000x reference)
import os
os.environ.setdefault("JAX_PLATFORMS", "")
import numpy as np

N_CORES = 8
B = 4096
F = 2048
RPC = 512
MB = 4
ALPHA = 100.0
BETA = 0.5
K_NN = 11
EPS = 1e-12

LAST_EXEC_NS = None
_NC_CACHE = {}


def _host_glue(descriptors, centroids):
    import jax
    import jax.numpy as jnp
    cpu = jax.devices("cpu")[0]
    with jax.default_device(cpu):
        x = jnp.asarray(descriptors, dtype=jnp.float32)
        c = jnp.asarray(centroids, dtype=jnp.float32)
        x = x / jnp.maximum(jnp.linalg.norm(x, axis=-1, keepdims=True), EPS)
        logits = (2.0 * ALPHA * jnp.einsum('bnd,kd->bkn', x, c)
                  - ALPHA * jnp.linalg.norm(c, axis=1)[None, :, None])
        a = jax.nn.softmax(logits, axis=1)
        vlad = (jnp.einsum('bkn,bnd->bkd', a, x)
                - jnp.sum(a, axis=-1)[..., None] * c[None])
        vlad = vlad / jnp.maximum(jnp.linalg.norm(vlad, axis=-1, keepdims=True), EPS)
        vlad = vlad.reshape(vlad.shape[0], -1)
        g = vlad / jnp.maximum(jnp.linalg.norm(vlad, axis=-1, keepdims=True), EPS)
        sq = (jnp.sum(g * g, -1)[:, None] + jnp.sum(g * g, -1)[None, :]
              - 2.0 * g @ g.T)
        dis = jnp.sqrt(jnp.maximum(sq, EPS))
        _, idx = jax.lax.top_k(-dis, K_NN)
        nd = g[idx]
        w = jnp.sum(nd * g[:, None, :], axis=-1)
        scale = jnp.concatenate([jnp.ones((1,), g.dtype),
                                 jnp.full((K_NN - 1,), BETA, g.dtype)])
        w = w * scale[None, :]
        den = jnp.sum(w, axis=1)
        g_np = np.asarray(g, dtype=np.float32)
        idx_np = np.asarray(idx)
        w_np = np.asarray(w, dtype=np.float32)
        den_np = np.asarray(den, dtype=np.float32)
    W = np.zeros((B, B), dtype=np.float32)
    np.add.at(W, (np.arange(B)[:, None], idx_np), w_np)
    return g_np, W, den_np


def _build():
    import concourse.bass as bass  # noqa: F401
    import concourse.bacc as bacc
    import concourse.mybir as mybir
    import concourse.tile as tile

    DT = mybir.dt.float32
    AF = mybir.ActivationFunctionType
    OP = mybir.AluOpType

    nc = bacc.Bacc("TRN2", target_bir_lowering=False, debug=False,
                   num_devices=N_CORES)
    wT_d = nc.dram_tensor("wT", [B, RPC], DT, kind="ExternalInput")
    gfull = nc.dram_tensor("gfull", [B, F], DT, kind="ExternalInput")
    winv_d = nc.dram_tensor("winv", [128, MB], DT, kind="ExternalInput")
    iden_d = nc.dram_tensor("iden", [128, 128], DT, kind="ExternalInput")
    ones_d = nc.dram_tensor("onesr", [1, 512], DT, kind="ExternalInput")
    out_d = nc.dram_tensor("out", [RPC, B], DT, kind="ExternalOutput")

    with tile.TileContext(nc) as tc:
        with tc.tile_pool(name="dram", bufs=1, space="DRAM") as dram, \
             tc.tile_pool(name="pers", bufs=1) as pers, \
             tc.tile_pool(name="stream", bufs=3) as stream, \
             tc.tile_pool(name="outp", bufs=4) as outp, \
             tc.tile_pool(name="psA", bufs=1, space="PSUM") as psA, \
             tc.tile_pool(name="psT", bufs=2, space="PSUM") as psT, \
             tc.tile_pool(name="psN", bufs=1, space="PSUM") as psN:

            idsb = pers.tile([128, 128], DT)
            nc.sync.dma_start(idsb[:], iden_d[:])
            winv = pers.tile([128, MB], DT)
            nc.sync.dma_start(winv[:], winv_d[:])
            onesb = pers.tile([1, 512], DT)
            nc.sync.dma_start(onesb[:], ones_d[:])
            wTsb = pers.tile([128, 32, 512], DT)
            for jc in range(32):
                nc.sync.dma_start(wTsb[:, jc, :],
                                  wT_d[128 * jc:128 * jc + 128, :])

            ref = [pers.tile([128, F], DT, name=f"ref{i}") for i in range(MB)]
            rT = pers.tile([128, 16, 512], DT)
            sq = pers.tile([128, F], DT)
            nrsb = pers.tile([128, MB], DT)
            nrT = pers.tile([4, 128], DT)
            nrjs = pers.tile([1, B], DT)

            # refine: refined = (W @ gfull) * winv, per 512-col feature tile
            for ft in range(4):
                ps4 = [psA.tile([128, 512], DT, name=f"psr{b}")
                       for b in range(MB)]
                for jc in range(32):
                    rt = stream.tile([128, 512], DT)
                    nc.sync.dma_start(
                        rt[:], gfull[128 * jc:128 * jc + 128,
                                     512 * ft:512 * ft + 512])
                    for b in range(MB):
                        nc.tensor.matmul(
                            ps4[b][:],
                            wTsb[:, jc, 128 * b:128 * b + 128],
                            rt[:], start=(jc == 0), stop=(jc == 31))
                for b in range(MB):
                    nc.scalar.activation(
                        ref[b][:, 512 * ft:512 * ft + 512], ps4[b][:],
                        AF.Copy, scale=winv[:, b:b + 1])

            # nr = ||refined||^2 per row; rT = refined^T
            for b in range(MB):
                nc.scalar.activation(sq[:], ref[b][:], AF.Square,
                                     accum_out=nrsb[:, b:b + 1])
                for q in range(16):
                    pt = psT.tile([128, 128], DT)
                    nc.tensor.transpose(
                        pt[:], ref[b][:, 128 * q:128 * q + 128], idsb[:])
                    nc.vector.tensor_copy(rT[:, q, 128 * b:128 * b + 128],
                                          pt[:])

            pn = psN.tile([4, 128], DT)
            nc.tensor.transpose(pn[:], nrsb[:], idsb[:])
            nc.vector.tensor_scalar_mul(nrT[:], pn[:], -0.5)
            nrm = [pers.tile([1, 128], DT, name=f"nrm{i}") for i in range(MB)]
            for b in range(MB):
                nc.sync.dma_start(nrm[b][:], nrT[b:b + 1, :])

            rT_dram = dram.tile([F, RPC], DT)
            agT = dram.tile([N_CORES * F, RPC], DT, addr_space="Shared")
            nr_in = dram.tile([RPC, 1], DT)
            nr_all = dram.tile([B, 1], DT, addr_space="Shared")
            for q in range(16):
                nc.sync.dma_start(rT_dram[128 * q:128 * q + 128, :],
                                  rT[:, q, :])
            nc.sync.dma_start(nr_in[:], nrT[:])
            nc.gpsimd.collective_compute(
                "AllGather", OP.bypass,
                replica_groups=[list(range(N_CORES))],
                ins=[rT_dram[:]], outs=[agT[:]])
            nc.gpsimd.collective_compute(
                "AllGather", OP.bypass,
                replica_groups=[list(range(N_CORES))],
                ins=[nr_in[:]], outs=[nr_all[:]])
            nc.sync.dma_start(nrjs[:], nr_all[:].rearrange("a b -> b a"))

            # final gram + overlap: psum = r_m . r_j - 0.5 nr_m - 0.5 nr_j
            # out = 1 - 0.5*sqrt(max(-2*psum, 1e-12))
            for cp in range(N_CORES):
                ps4 = [psA.tile([128, 512], DT, name=f"psr{b}")
                       for b in range(MB)]
                for fc in range(16):
                    rt = stream.tile([128, 512], DT)
                    base = 2048 * cp + 128 * fc
                    nc.sync.dma_start(rt[:], agT[base:base + 128, :])
                    for b in range(MB):
                        nc.tensor.matmul(
                            ps4[b][:], rT[:, fc, 128 * b:128 * b + 128],
                            rt[:], start=(fc == 0), stop=False)
                for b in range(MB):
                    nc.tensor.matmul(ps4[b][:], nrm[b][:],
                                     onesb[:, 0:512], start=False,
                                     stop=False, skip_group_check=True)
                    nc.tensor.matmul(ps4[b][:], onesb[:, 0:128],
                                     nrjs[:, 512 * cp:512 * cp + 512],
                                     start=False, stop=True,
                                     skip_group_check=True)
                for b in range(MB):
                    t1 = outp.tile([128, 512], DT)
                    t2 = outp.tile([128, 512], DT)
                    nc.vector.tensor_scalar(t1[:], ps4[b][:], -2.0, 1e-12,
                                            OP.mult, OP.max)
                    nc.scalar.sqrt(t2[:], t1[:])
                    nc.vector.tensor_scalar(t1[:], t2[:], -0.5, 1.0,
                                            OP.mult, OP.add)
                    nc.sync.dma_start(
                        out_d[128 * b:128 * b + 128,
                              512 * cp:512 * cp + 512], t1[:])
    nc.compile()
    return nc


def kernel(descriptors: np.ndarray, centroids: np.ndarray) -> np.ndarray:
    global LAST_EXEC_NS
    from concourse.bass_utils import run_bass_kernel_spmd

    g, W, den = _host_glue(descriptors, centroids)

    if "nc" not in _NC_CACHE:
        _NC_CACHE["nc"] = _build()
    nc = _NC_CACHE["nc"]

    eye = np.eye(128, dtype=np.float32)
    ones = np.ones((1, 512), dtype=np.float32)
    gfull = np.ascontiguousarray(g, dtype=np.float32)
    in_maps = []
    for c in range(N_CORES):
        wT_c = np.ascontiguousarray(W[512 * c:512 * c + 512, :].T)
        winv_c = np.ascontiguousarray(
            (1.0 / den[512 * c:512 * c + 512]).astype(np.float32)
            .reshape(MB, 128).T)
        in_maps.append({"wT": wT_c, "gfull": gfull, "winv": winv_c,
                        "iden": eye, "onesr": ones})

    import time
    t0 = time.perf_counter_ns()
    r = run_bass_kernel_spmd(nc, in_maps, list(range(N_CORES)), trace=False)
    t1 = time.perf_counter_ns()
    LAST_EXEC_NS = getattr(r, "exec_time_ns", None) or (t1 - t0)

    out = np.concatenate([r.results[i]["out"] for i in range(N_CORES)],
                         axis=0).astype(np.float32)
    np.fill_diagonal(out, 0.0)
    return out



# revision 16
# speedup vs baseline: 5.3983x; 5.3983x over previous
import os
os.environ.setdefault("JAX_PLATFORMS", "")
import numpy as np

N_CORES = 8
B = 4096
F = 2048
RPC = 512          # rows per core
NITB = RPC // 128  # 4 row-tiles of 128 per core
NJB = B // 512     # 8 column slabs of 512
NFC = F // 128     # 16 feature chunks of 128
ALPHA = 100.0
EPS = 1e-12

LAST_EXEC_NS = None
_NC_CACHE = {}


def _host_netvlad(descriptors, centroids):
    """NetVLAD pooling on host (tiny: ~2 GFLOP). All B x B work is on-device."""
    import jax
    import jax.numpy as jnp
    cpu = jax.devices("cpu")[0]
    with jax.default_device(cpu):
        x = jnp.asarray(descriptors, dtype=jnp.float32)
        c = jnp.asarray(centroids, dtype=jnp.float32)
        x = x / jnp.maximum(jnp.linalg.norm(x, axis=-1, keepdims=True), EPS)
        logits = (2.0 * ALPHA * jnp.einsum('bnd,kd->bkn', x, c)
                  - ALPHA * jnp.linalg.norm(c, axis=1)[None, :, None])
        a = jax.nn.softmax(logits, axis=1)
        vlad = (jnp.einsum('bkn,bnd->bkd', a, x)
                - jnp.sum(a, axis=-1)[..., None] * c[None])
        vlad = vlad / jnp.maximum(
            jnp.linalg.norm(vlad, axis=-1, keepdims=True), EPS)
        vlad = vlad.reshape(vlad.shape[0], -1)
        g = vlad / jnp.maximum(jnp.linalg.norm(vlad, axis=-1, keepdims=True),
                               EPS)
        gb = jnp.asarray(g, dtype=jnp.bfloat16)
        gTb = jnp.asarray(g.T, dtype=jnp.bfloat16)
        return np.asarray(gb), np.asarray(gTb)


def _build():
    import concourse.bass as bass  # noqa: F401
    import concourse.bacc as bacc
    import concourse.mybir as mybir
    import concourse.tile as tile

    F32 = mybir.dt.float32
    BF = mybir.dt.bfloat16
    AF = mybir.ActivationFunctionType
    OP = mybir.AluOpType
    AX = mybir.AxisListType

    nc = bacc.Bacc("TRN2", target_bir_lowering=False, debug=False,
                   num_devices=N_CORES)
    # replicated inputs
    gT_d = nc.dram_tensor("gT", [F, B], BF, kind="ExternalInput")
    g_d = nc.dram_tensor("g", [B, F], BF, kind="ExternalInput")
    idb_d = nc.dram_tensor("idb", [128, 128], BF, kind="ExternalInput")
    oneb_d = nc.dram_tensor("oneb", [1, 512], BF, kind="ExternalInput")
    # per-core inputs
    gTo_d = nc.dram_tensor("gTo", [F, RPC], BF, kind="ExternalInput")
    gown_d = nc.dram_tensor("gown", [RPC, F], BF, kind="ExternalInput")
    out_d = nc.dram_tensor("out", [RPC, B], F32, kind="ExternalOutput")

    with tile.TileContext(nc) as tc:
        with tc.tile_pool(name="dram", bufs=1, space="DRAM") as dram, \
             tc.tile_pool(name="pers", bufs=1) as pers, \
             tc.tile_pool(name="slab", bufs=2) as slab, \
             tc.tile_pool(name="sm", bufs=2) as sm, \
             tc.tile_pool(name="scr", bufs=3) as scr:

            idb = pers.tile([128, 128], BF)
            nc.sync.dma_start(idb[:], idb_d[:])
            btwo = pers.tile([128, 1], F32)
            nc.vector.memset(btwo[:], 2.0)
            oneb = pers.tile([1, 512], BF)
            nc.sync.dma_start(oneb[:], oneb_d[:])
            # own G^T (lhsT for S gram): [128f, fc, it]
            gto = pers.tile([128, NFC, RPC], BF)
            nc.sync.dma_start(
                gto[:], gTo_d[:].rearrange("(q p) j -> p q j", p=128))

            # M^T for refine lhsT: [128j, jc, it]
            MT = pers.tile([128, B // 128, RPC], BF)
            rden = pers.tile([128, NITB], F32)
            nrow = pers.tile([1, 512], BF)
            r2p = pers.tile([128, NITB, 4], F32)

            rtag = dram.tile([F + 1, RPC], BF)
            rtag_all = dram.tile([N_CORES * (F + 1), RPC], BF,
                                 addr_space="Shared")

            # ---- Phase 1: S = G_own @ G_all^T (bf16, psum f32) ----
            with tc.tile_pool(name="spool", bufs=1) as spool, \
                 tc.tile_pool(name="psS", bufs=1, space="PSUM") as psS, \
                 tc.tile_pool(name="psT", bufs=2, space="PSUM") as psT:
                # S rows for all 4 row-tiles (f32, whole-row for topk)
                S = [spool.tile([128, NJB, 512], F32, name=f"S{i}")
                     for i in range(NITB)]
                for jb in range(NJB):
                    ps = [psS.tile([128, 512], F32, name=f"s{i}")
                          for i in range(NITB)]
                    for h in range(2):
                        gta = slab.tile([128, NFC // 2, 512], BF)
                        nc.sync.dma_start(
                            gta[:],
                            gT_d[1024 * h:1024 * h + 1024,
                                 512 * jb:512 * jb + 512]
                            .rearrange("(q p) j -> p q j", p=128))
                        for itb in range(NITB):
                            for fc in range(NFC // 2):
                                nc.tensor.matmul(
                                    ps[itb][:],
                                    gto[:, 8 * h + fc,
                                        128 * itb:128 * itb + 128],
                                    gta[:, fc, :],
                                    start=(h == 0 and fc == 0),
                                    stop=(h == 1 and fc == NFC // 2 - 1))
                    for itb in range(NITB):
                        nc.scalar.copy(S[itb][:, jb, :], ps[itb][:])

                # ---- Phase 2: top-11 threshold + mask + M^T ----
                for itb in range(NITB):
                    srow = S[itb][:].rearrange("p a b -> p (a b)")
                    mx1 = scr.tile([128, 8], F32)
                    nc.vector.max(mx1[:], srow)
                    # slice-wise match_replace (values are globally unique),
                    # then global top-8 of per-slice top-8s = ranks 9..16
                    mxs = scr.tile([128, NJB, 8], F32)
                    for jb in range(NJB):
                        srj = scr.tile([128, 512], F32)
                        nc.vector.match_replace(
                            srj[:], mx1[:], S[itb][:, jb, :], -3.0)
                        nc.vector.max(mxs[:, jb, :], srj[:])
                    mx2 = scr.tile([128, 8], F32)
                    nc.vector.max(mx2[:],
                                  mxs[:].rearrange("p a b -> p (a b)"))
                    # t11 = 3rd largest of the second batch = 11th overall
                    racc = scr.tile([128, NJB], F32)
                    for jb in range(NJB):
                        mjb = sm.tile([128, 512], BF)
                        nc.vector.scalar_tensor_tensor(
                            mjb[:], S[itb][:, jb, :], mx2[:, 2:3],
                            S[itb][:, jb, :], OP.is_ge, OP.mult,
                            accum_out=racc[:, jb:jb + 1])
                        for q in range(4):
                            pt = psT.tile([128, 128], BF)
                            nc.tensor.transpose(
                                pt[:], mjb[:, 128 * q:128 * q + 128], idb[:])
                            nc.vector.tensor_copy(
                                MT[:, 4 * jb + q, 128 * itb:128 * itb + 128],
                                pt[:])
                    dsum = scr.tile([128, 1], F32)
                    nc.vector.reduce_sum(dsum[:], racc[:], axis=AX.X)
                    den = scr.tile([128, 1], F32)
                    nc.vector.tensor_scalar_add(den[:], dsum[:], 1.0)
                    nc.vector.reciprocal(rden[:, itb:itb + 1], den[:])

            # ---- Phase 3: refine R = rden * (M @ G_all + G_own) ----
            rpool_cm = tc.tile_pool(name="rpool", bufs=1)
            rpool = rpool_cm.__enter__()
            RT = rpool.tile([128, NFC, RPC], BF)
            Rsb = rpool.tile([128, 8, 512], BF)
            for half in range(2):
                fbase = 1024 * half
                with tc.tile_pool(name=f"psR{half}", bufs=1,
                                  space="PSUM") as psR:
                    psr = [psR.tile([128, 512], F32, name=f"psr{i}")
                           for i in range(8)]
                    for jc in range(B // 128):
                        gch = scr.tile([128, 1024], BF)
                        nc.sync.dma_start(
                            gch[:],
                            g_d[128 * jc:128 * jc + 128,
                                fbase:fbase + 1024])
                        for itb in range(NITB):
                            for fb in range(2):
                                p = psr[4 * fb + itb]
                                if jc == 0:
                                    go = scr.tile([128, 512], BF)
                                    nc.sync.dma_start(
                                        go[:],
                                        gown_d[128 * itb:128 * itb + 128,
                                               fbase + 512 * fb:
                                               fbase + 512 * fb + 512])
                                    nc.tensor.matmul(
                                        p[:], idb[:], go[:],
                                        start=True, stop=False)
                                nc.tensor.matmul(
                                    p[:],
                                    MT[:, jc, 128 * itb:128 * itb + 128],
                                    gch[:, 512 * fb:512 * fb + 512],
                                    start=False, stop=(jc == B // 128 - 1))
                    # scale rows by rden, accumulate r2
                    for itb in range(NITB):
                        for fb in range(2):
                            p = psr[4 * fb + itb]
                            rb = Rsb[:, 4 * fb + itb, :]
                            nc.scalar.activation(
                                rb, p[:], AF.Copy,
                                scale=rden[:, itb:itb + 1])
                            junk = scr.tile([128, 512], F32)
                            nc.vector.scalar_tensor_tensor(
                                junk[:], rb, 1.0, rb,
                                OP.mult, OP.mult,
                                accum_out=r2p[:, itb, 2 * half + fb:
                                              2 * half + fb + 1])
                # build R^T for this half (psR closed, psum free)
                with tc.tile_pool(name=f"psT2{half}", bufs=2,
                                  space="PSUM") as psT2:
                    for itb in range(NITB):
                        for fb in range(2):
                            for q in range(4):
                                fcg = (fbase + 512 * fb) // 128 + q
                                pt = psT2.tile([128, 128], BF)
                                nc.tensor.transpose(
                                    pt[:],
                                    Rsb[:, 4 * fb + itb,
                                        128 * q:128 * q + 128],
                                    idb[:])
                                nc.vector.tensor_copy(
                                    RT[:, fcg,
                                       128 * itb:128 * itb + 128],
                                    pt[:])

            # nrow = -0.5*(r2-1) as [1, 512] bf16
            with tc.tile_pool(name="psT3", bufs=1, space="PSUM") as psT3:
                r2 = scr.tile([128, NITB], F32)
                nc.vector.reduce_sum(r2[:], r2p[:], axis=AX.X)
                nr05 = scr.tile([128, NITB], BF)
                nc.vector.tensor_scalar(nr05[:], r2[:], -1.0, -0.5,
                                        OP.add, OP.mult)
                pn = psT3.tile([NITB, 128], BF)
                nc.tensor.transpose(pn[:], nr05[:], idb[:])
                nrs = scr.tile([NITB, 128], BF)
                nc.vector.tensor_copy(nrs[:], pn[:])
                nc.sync.dma_start(
                    rtag[F:F + 1, :].rearrange("o (a b) -> (o a) b",
                                               a=NITB), nrs[:])
                nc.sync.dma_start(nrow[:], rtag[F:F + 1, :])

            # publish R^T and allgather
            nc.sync.dma_start(
                rtag[0:F, :].rearrange("(q p) j -> p q j", p=128), RT[:])
            nc.gpsimd.collective_compute(
                "AllGather", OP.bypass,
                replica_groups=[list(range(N_CORES))],
                ins=[rtag[:]], outs=[rtag_all[:]])

            # ---- Phase 4: final gram + overlap epilogue ----
            with tc.tile_pool(name="psF", bufs=1, space="PSUM") as psF:
                for cj in range(N_CORES):
                    base = (F + 1) * cj
                    nrj = scr.tile([1, 512], BF)
                    nc.sync.dma_start(nrj[:],
                                      rtag_all[base + F:base + F + 1, :])
                    ps4 = [psF.tile([128, 512], F32, name=f"f{i}")
                           for i in range(NITB)]
                    for h in range(2):
                        rta = slab.tile([128, NFC // 2, 512], BF)
                        nc.sync.dma_start(
                            rta[:],
                            rtag_all[base + 1024 * h:base + 1024 * h + 1024,
                                     :]
                            .rearrange("(q p) j -> p q j", p=128))
                        for itb in range(NITB):
                            for fc in range(NFC // 2):
                                nc.tensor.matmul(
                                    ps4[itb][:],
                                    RT[:, 8 * h + fc,
                                       128 * itb:128 * itb + 128],
                                    rta[:, fc, :],
                                    start=(h == 0 and fc == 0), stop=False)
                    for itb in range(NITB):
                        ps = ps4[itb]
                        nc.tensor.matmul(ps[:],
                                         nrow[:, 128 * itb:128 * itb + 128],
                                         oneb[:], start=False, stop=False)
                        nc.tensor.matmul(ps[:], oneb[:, 0:128], nrj[:],
                                         start=False, stop=True)
                        t0 = scr.tile([128, 512], F32)
                        nc.vector.tensor_scalar(t0[:], ps[:], 1.0, -2.0,
                                                OP.min, OP.mult)
                        t1 = scr.tile([128, 512], F32)
                        nc.scalar.activation(t1[:], t0[:], AF.Sqrt,
                                             bias=btwo[:], scale=1.0)
                        t2 = scr.tile([128, 512], F32)
                        nc.vector.tensor_scalar(t2[:], t1[:], -0.5, 1.0,
                                                OP.mult, OP.add)
                        nc.sync.dma_start(
                            out_d[128 * itb:128 * itb + 128,
                                  512 * cj:512 * cj + 512], t2[:])
            rpool_cm.__exit__(None, None, None)
    nc.compile()
    return nc


def _in_maps(gb, gTb):
    import ml_dtypes
    eye = np.eye(128, dtype=ml_dtypes.bfloat16)
    ones = np.ones((1, 512), dtype=ml_dtypes.bfloat16)
    maps = []
    for c in range(N_CORES):
        maps.append({
            "gT": gTb, "g": gb, "idb": eye, "oneb": ones,
            "gTo": np.ascontiguousarray(gTb[:, RPC * c:RPC * c + RPC]),
            "gown": np.ascontiguousarray(gb[RPC * c:RPC * c + RPC, :]),
        })
    return maps


def kernel(descriptors: np.ndarray, centroids: np.ndarray) -> np.ndarray:
    global LAST_EXEC_NS
    from concourse.bass_utils import run_bass_kernel_spmd

    gb, gTb = _host_netvlad(descriptors, centroids)

    if "nc" not in _NC_CACHE:
        _NC_CACHE["nc"] = _build()
    nc = _NC_CACHE["nc"]

    in_maps = _in_maps(gb, gTb)

    import time
    t0 = time.perf_counter_ns()
    r = run_bass_kernel_spmd(nc, in_maps, list(range(N_CORES)), trace=False)
    t1 = time.perf_counter_ns()
    LAST_EXEC_NS = getattr(r, "exec_time_ns", None) or (t1 - t0)

    out = np.concatenate([r.results[i]["out"] for i in range(N_CORES)],
                         axis=0).astype(np.float32)
    np.fill_diagonal(out, 0.0)
    return out
